# revision 14
# baseline (speedup 1.0000x reference)
"""RNN-T JointNet kernel for 8 Trainium2 NeuronCores.

out[b,t,u,:] = gelu_tanh(enc[b,t]@We + dec[b,u]@Wd + b1) @ Wfc

Sharding: flatten (B=4, T=512) -> 2048 rows, 256 contiguous rows per core.
Core c handles batch b=c//2, time slice t0=(c%2)*256 .. +256. Each core only
needs its own enc slice and one batch's dec.

Per-core layout (all fp32):
  - encT (D=256, TC=256), decT (D=256, U=128): host-transposed so the first
    matmuls produce pe/pd with H on partitions, t/u on the free dim.
  - peb[h, t] = enc@We + b1 (4 h-tiles of (128, 256) in SBUF)
  - pd[h, u]  = dec@Wd      (4 h-tiles of (128, 128) in SBUF)
  - main loop over groups of TB=8 t's:
      DVE:  tmp[h, tb, u] = pd[h, u] + peb[h, t0+tb]   (broadcast APs)
      ACT:  hact = gelu_tanh(tmp)
      PE :  out_psum(u=128, v=512) = sum_ht hact[ht][:, tb]ᵀ-block @ Wfc[ht]
            (hact tile is the stationary operand, Wfc streams, N=512)
      DMA:  out_psum -> out[t] (contiguous 256 KiB)
"""

import sys

import numpy as np

sys.path.insert(0, "/opt/trn_rl_repo")

import concourse.bacc as bacc
import concourse.bass as bass
import concourse.mybir as mybir
import concourse.tile as tile
from concourse.bass_utils import run_bass_kernel_spmd

B, T, U, D, H, V = 4, 512, 128, 256, 512, 512
NCORES = 8
TC = (B * T) // NCORES  # 256 t-rows per core
TB = 8  # t's per main-loop group

_PROGRAM = None
LAST_RESULT = None


def _build():
    global _PROGRAM
    if _PROGRAM is not None:
        return _PROGRAM

    f32 = mybir.dt.float32
    # Bacc (not raw Bass): its compile() pipeline moves matmul waits onto
    # ldweights and splits >1-wait instructions via event semaphores —
    # walrus rejects matmuls carrying 2 sync waits otherwise.
    nc = bacc.Bacc("TRN2", target_bir_lowering=False)

    encT_d = nc.declare_dram_parameter("encT", (D, TC), f32, isOutput=False)
    decT_d = nc.declare_dram_parameter("decT", (D, U), f32, isOutput=False)
    w1_d = nc.declare_dram_parameter("W1", (2 * D, H), f32, isOutput=False)
    b1_d = nc.declare_dram_parameter("b1", (H, 1), f32, isOutput=False)
    wfc_d = nc.declare_dram_parameter("Wfc", (H, V), f32, isOutput=False)
    out_d = nc.declare_dram_parameter("out", (TC, U, V), f32, isOutput=True)

    GELU = mybir.ActivationFunctionType.Gelu_apprx_tanh
    IDENT = mybir.ActivationFunctionType.Identity
    # Same bits as fp32, but the PE streams it at 1 cycle/row (vs 4 for
    # plain fp32) when the moving free dim is >= 256.
    F32R = mybir.dt.float32r

    with tile.TileContext(nc) as tc:
        with (
            tc.tile_pool(name="const", bufs=1) as cpool,
            tc.tile_pool(name="work", bufs=2) as wpool,
            tc.tile_pool(name="outsb", bufs=2) as osb_pool,
            tc.tile_pool(name="pro_ps", bufs=2, space="PSUM") as pro_ps,
            tc.tile_pool(name="out_ps", bufs=4, space="PSUM") as out_ps_pool,
        ):
            # W1 row-block i (128 rows of the 512-row input dim) lives at
            # cols [i*H, (i+1)*H). Blocks 0,1 = We; blocks 2,3 = Wd.
            # Tiles that only feed matmuls are float32r so the PE streams
            # them at 1 cycle/row; the BIR verifier requires the producer
            # (DMA/ACT) output dtype to be f32r as well.
            w1_sb = cpool.tile([128, 4 * H], F32R)
            wfc_sb = cpool.tile([128, 4 * V], F32R)  # block ht = Wfc[ht*128:...]
            b1_sb = cpool.tile([128, 4], f32)  # col ht = b1[ht*128:(ht+1)*128]
            encT_sb = cpool.tile([128, 2 * TC], F32R)
            decT_sb = cpool.tile([128, 2 * U], F32R)
            peb_sb = cpool.tile([128, 4 * TC], f32)
            pd_sb = cpool.tile([128, 4 * U], f32)

            # One DMA per SBUF tile (3D APs): keeps the per-instruction sync
            # wait count low (walrus rejects >N waits on a matmul) and the
            # transfers large.
            nc.sync.dma_start(
                w1_sb.rearrange("p (i h) -> p i h", i=4),
                w1_d[:, :].rearrange("(i p) h -> p i h", p=128).bitcast(F32R),
            )
            nc.sync.dma_start(
                wfc_sb.rearrange("p (i v) -> p i v", i=4),
                wfc_d[:, :].rearrange("(i p) v -> p i v", p=128).bitcast(F32R),
            )
            nc.sync.dma_start(
                b1_sb, b1_d[:, :].rearrange("(i p) o -> p (i o)", p=128)
            )
            nc.sync.dma_start(
                encT_sb.rearrange("p (i t) -> p i t", i=2),
                encT_d[:, :].rearrange("(i p) t -> p i t", p=128).bitcast(F32R),
            )
            nc.sync.dma_start(
                decT_sb.rearrange("p (i u) -> p i u", i=2),
                decT_d[:, :].rearrange("(i p) u -> p i u", p=128).bitcast(F32R),
            )

            # Prologue: pe[h,t] = enc@We ; pdb[h,u] = dec@Wd + b1
            for ht in range(4):
                pe_ps = pro_ps.tile([128, TC], f32)
                for di in range(2):
                    nc.tensor.matmul(
                        pe_ps,
                        w1_sb[:, di * H + ht * 128 : di * H + (ht + 1) * 128],
                        encT_sb[:, di * TC : (di + 1) * TC],
                        start=(di == 0),
                        stop=(di == 1),
                    )
                nc.scalar.copy(peb_sb[:, ht * TC : (ht + 1) * TC], pe_ps)
                pd_ps = pro_ps.tile([128, U], f32)
                for di in range(2):
                    nc.tensor.matmul(
                        pd_ps,
                        w1_sb[:, (2 + di) * H + ht * 128 : (2 + di) * H + (ht + 1) * 128],
                        decT_sb[:, di * U : (di + 1) * U],
                        start=(di == 0),
                        stop=(di == 1),
                    )
                nc.scalar.activation(
                    pd_sb[:, ht * U : (ht + 1) * U],
                    pd_ps,
                    IDENT,
                    bias=b1_sb[:, ht : ht + 1],
                )

            # Main loop over u (in blocks of UB): ACT fuses the pd[:,u] add
            # into the GELU as a per-partition bias; h_u (h on partitions, t
            # free) feeds the PE as the stationary operand; DVE bounces
            # PSUM->SBUF staging; one DMA per (ts, u-block) stores
            # (128 t, UB u, 512 v) slabs — 16 KiB contiguous per t row.
            UB = 8
            for ub in range(U // UB):
                stages = [
                    osb_pool.tile(
                        [128, UB * V], f32, tag=f"st{ts}", name=f"stage{ts}"
                    )
                    for ts in range(TC // 128)
                ]
                for j in range(UB):
                    u = ub * UB + j
                    hts = []
                    for ht in range(4):
                        hact = wpool.tile([128, TC], F32R, tag=f"h{ht}")
                        nc.scalar.activation(
                            hact,
                            peb_sb[:, ht * TC : (ht + 1) * TC],
                            GELU,
                            bias=pd_sb[:, ht * U + u : ht * U + u + 1],
                        )
                        hts.append(hact)
                    for ts in range(TC // 128):
                        ops = out_ps_pool.tile([128, V], f32)
                        for ht in range(4):
                            nc.tensor.matmul(
                                ops,
                                hts[ht][:, ts * 128 : (ts + 1) * 128],
                                wfc_sb[:, ht * V : (ht + 1) * V],
                                start=(ht == 0),
                                stop=(ht == 3),
                            )
                        nc.vector.tensor_copy(
                            stages[ts][:, j * V : (j + 1) * V], ops
                        )
                for ts in range(TC // 128):
                    nc.sync.dma_start(
                        out_d[ts * 128 : (ts + 1) * 128, ub * UB : (ub + 1) * UB, :],
                        stages[ts].rearrange("p (j v) -> p j v", j=UB),
                    )

    nc.compile()
    _PROGRAM = nc
    return nc


def kernel(enc, dec, W1, b1, Wfc):
    global LAST_RESULT
    nc = _build()
    enc = np.asarray(enc, dtype=np.float32)
    dec = np.asarray(dec, dtype=np.float32)
    W1 = np.ascontiguousarray(np.asarray(W1, dtype=np.float32))
    b1 = np.ascontiguousarray(np.asarray(b1, dtype=np.float32).reshape(H, 1))
    Wfc = np.ascontiguousarray(np.asarray(Wfc, dtype=np.float32))

    in_maps = []
    for c in range(NCORES):
        b, t0 = c // 2, (c % 2) * TC
        in_maps.append(
            {
                "encT": np.ascontiguousarray(enc[b, t0 : t0 + TC, :].T),
                "decT": np.ascontiguousarray(dec[b].T),
                "W1": W1,
                "b1": b1,
                "Wfc": Wfc,
            }
        )

    LAST_RESULT = run_bass_kernel_spmd(nc, in_maps, list(range(NCORES)))

    out = np.empty((B, T, U, V), np.float32)
    for c in range(NCORES):
        b, t0 = c // 2, (c % 2) * TC
        out[b, t0 : t0 + TC] = LAST_RESULT.results[c]["out"]
    return out



# revision 22
# speedup vs baseline: 1.0894x; 1.0894x over previous
"""RNN-T JointNet kernel for 8 Trainium2 NeuronCores.

out[b,t,u,:] = gelu_tanh(enc[b,t]@We + dec[b,u]@Wd + b1) @ Wfc

Sharding: flatten (B=4, T=512) -> 2048 rows, 256 contiguous rows per core.
Core c handles batch b=c//2, time slice t0=(c%2)*256 .. +256. Each core only
needs its own enc slice and one batch's dec.

Per-core layout (all fp32):
  - encT (D=256, TC=256), decT (D=256, U=128): host-transposed so the first
    matmuls produce pe/pd with H on partitions, t/u on the free dim.
  - peb[h, t] = enc@We + b1 (4 h-tiles of (128, 256) in SBUF)
  - pd[h, u]  = dec@Wd      (4 h-tiles of (128, 128) in SBUF)
  - main loop over groups of TB=8 t's:
      DVE:  tmp[h, tb, u] = pd[h, u] + peb[h, t0+tb]   (broadcast APs)
      ACT:  hact = gelu_tanh(tmp)
      PE :  out_psum(u=128, v=512) = sum_ht hact[ht][:, tb]ᵀ-block @ Wfc[ht]
            (hact tile is the stationary operand, Wfc streams, N=512)
      DMA:  out_psum -> out[t] (contiguous 256 KiB)
"""

import sys

import numpy as np

sys.path.insert(0, "/opt/trn_rl_repo")

import concourse.bacc as bacc
import concourse.bass as bass
import concourse.mybir as mybir
import concourse.tile as tile
from concourse.bass_utils import run_bass_kernel_spmd

B, T, U, D, H, V = 4, 512, 128, 256, 512, 512
NCORES = 8
TC = (B * T) // NCORES  # 256 t-rows per core
TB = 8  # t's per main-loop group

_PROGRAM = None
LAST_RESULT = None


def _build():
    global _PROGRAM
    if _PROGRAM is not None:
        return _PROGRAM

    f32 = mybir.dt.float32
    bf16 = mybir.dt.bfloat16
    # Bacc (not raw Bass): its compile() pipeline moves matmul waits onto
    # ldweights and splits >1-wait instructions via event semaphores —
    # walrus rejects matmuls carrying 2 sync waits otherwise.
    nc = bacc.Bacc("TRN2", target_bir_lowering=False)

    encT_d = nc.declare_dram_parameter("encT", (D, TC), f32, isOutput=False)
    decT_d = nc.declare_dram_parameter("decT", (D, U), f32, isOutput=False)
    w1_d = nc.declare_dram_parameter("W1", (2 * D, H), f32, isOutput=False)
    b1_d = nc.declare_dram_parameter("b1", (H, 1), f32, isOutput=False)
    wfc_d = nc.declare_dram_parameter("Wfc", (H, V), bf16, isOutput=False)
    out_d = nc.declare_dram_parameter("out", (TC, U, V), f32, isOutput=True)

    GELU = mybir.ActivationFunctionType.Gelu_apprx_tanh
    IDENT = mybir.ActivationFunctionType.Identity
    # Same bits as fp32, but the PE streams it at 1 cycle/row (vs 4 for
    # plain fp32) when the moving free dim is >= 256.
    F32R = mybir.dt.float32r

    with tile.TileContext(nc) as tc:
        with (
            tc.tile_pool(name="const", bufs=1) as cpool,
            tc.tile_pool(name="work", bufs=3) as wpool,
            tc.tile_pool(name="outsb", bufs=2) as osb_pool,
            tc.tile_pool(name="pro_ps", bufs=1, space="PSUM") as pro_ps,
            tc.tile_pool(name="out_ps", bufs=6, space="PSUM") as out_ps_pool,
        ):
            # W1 row-block i (128 rows of the 512-row input dim) lives at
            # cols [i*H, (i+1)*H). Blocks 0,1 = We; blocks 2,3 = Wd.
            # Tiles that only feed matmuls are float32r so the PE streams
            # them at 1 cycle/row; the BIR verifier requires the producer
            # (DMA/ACT) output dtype to be f32r as well.
            w1_sb = cpool.tile([128, 4 * H], F32R)
            wfc_sb = cpool.tile([128, 4 * V], bf16)  # block ht = Wfc[ht*128:...]
            b1_sb = cpool.tile([128, 4], f32)  # col ht = b1[ht*128:(ht+1)*128]
            encT_sb = cpool.tile([128, 2 * TC], F32R)
            decT_sb = cpool.tile([128, 2 * U], F32R)
            peb_sb = cpool.tile([128, 4 * TC], f32)
            pd_sb = cpool.tile([128, 4 * U], f32)

            # One DMA per SBUF tile (3D APs): keeps the per-instruction sync
            # wait count low (walrus rejects >N waits on a matmul) and the
            # transfers large.
            nc.sync.dma_start(
                w1_sb.rearrange("p (i h) -> p i h", i=4),
                w1_d[:, :].rearrange("(i p) h -> p i h", p=128).bitcast(F32R),
            )
            nc.sync.dma_start(
                wfc_sb.rearrange("p (i v) -> p i v", i=4),
                wfc_d[:, :].rearrange("(i p) v -> p i v", p=128),
            )
            nc.sync.dma_start(
                b1_sb, b1_d[:, :].rearrange("(i p) o -> p (i o)", p=128)
            )
            nc.sync.dma_start(
                encT_sb.rearrange("p (i t) -> p i t", i=2),
                encT_d[:, :].rearrange("(i p) t -> p i t", p=128).bitcast(F32R),
            )
            nc.sync.dma_start(
                decT_sb.rearrange("p (i u) -> p i u", i=2),
                decT_d[:, :].rearrange("(i p) u -> p i u", p=128).bitcast(F32R),
            )

            # Prologue: pe[h,t] = enc@We ; pdb[h,u] = dec@Wd + b1
            for ht in range(4):
                pe_ps = pro_ps.tile([128, TC], f32)
                for di in range(2):
                    nc.tensor.matmul(
                        pe_ps,
                        w1_sb[:, di * H + ht * 128 : di * H + (ht + 1) * 128],
                        encT_sb[:, di * TC : (di + 1) * TC],
                        start=(di == 0),
                        stop=(di == 1),
                    )
                nc.scalar.copy(peb_sb[:, ht * TC : (ht + 1) * TC], pe_ps)
                pd_ps = pro_ps.tile([128, U], f32)
                for di in range(2):
                    nc.tensor.matmul(
                        pd_ps,
                        w1_sb[:, (2 + di) * H + ht * 128 : (2 + di) * H + (ht + 1) * 128],
                        decT_sb[:, di * U : (di + 1) * U],
                        start=(di == 0),
                        stop=(di == 1),
                    )
                nc.scalar.activation(
                    pd_sb[:, ht * U : (ht + 1) * U],
                    pd_ps,
                    IDENT,
                    bias=b1_sb[:, ht : ht + 1],
                )

            # Main loop over u (in blocks of UB): ACT fuses the pd[:,u] add
            # into the GELU as a per-partition bias; h_u (h on partitions, t
            # free) feeds the PE as the stationary operand; DVE bounces
            # PSUM->SBUF staging; one DMA per (ts, u-block) stores
            # (128 t, UB u, 512 v) slabs — 16 KiB contiguous per t row.
            UB = 8
            for ub in range(U // UB):
                stages = [
                    osb_pool.tile(
                        [128, UB * V], f32, tag=f"st{ts}", name=f"stage{ts}"
                    )
                    for ts in range(TC // 128)
                ]
                for j in range(UB):
                    u = ub * UB + j
                    hts = []
                    for ht in range(4):
                        hact = wpool.tile([128, TC], bf16, tag=f"h{ht}")
                        nc.scalar.activation(
                            hact,
                            peb_sb[:, ht * TC : (ht + 1) * TC],
                            GELU,
                            bias=pd_sb[:, ht * U + u : ht * U + u + 1],
                        )
                        hts.append(hact)
                    for ts in range(TC // 128):
                        ops = out_ps_pool.tile([128, V], f32)
                        for ht in range(4):
                            nc.tensor.matmul(
                                ops,
                                hts[ht][:, ts * 128 : (ts + 1) * 128],
                                wfc_sb[:, ht * V : (ht + 1) * V],
                                start=(ht == 0),
                                stop=(ht == 3),
                            )
                        nc.vector.tensor_copy(
                            stages[ts][:, j * V : (j + 1) * V], ops
                        )
                for ts in range(TC // 128):
                    nc.sync.dma_start(
                        out_d[ts * 128 : (ts + 1) * 128, ub * UB : (ub + 1) * UB, :],
                        stages[ts].rearrange("p (j v) -> p j v", j=UB),
                    )

    nc.compile()
    _PROGRAM = nc
    return nc


def kernel(enc, dec, W1, b1, Wfc):
    global LAST_RESULT
    nc = _build()
    enc = np.asarray(enc, dtype=np.float32)
    dec = np.asarray(dec, dtype=np.float32)
    import ml_dtypes

    W1 = np.ascontiguousarray(np.asarray(W1, dtype=np.float32))
    b1 = np.ascontiguousarray(np.asarray(b1, dtype=np.float32).reshape(H, 1))
    Wfc = np.ascontiguousarray(
        np.asarray(Wfc, dtype=np.float32).astype(ml_dtypes.bfloat16)
    )

    in_maps = []
    for c in range(NCORES):
        b, t0 = c // 2, (c % 2) * TC
        in_maps.append(
            {
                "encT": np.ascontiguousarray(enc[b, t0 : t0 + TC, :].T),
                "decT": np.ascontiguousarray(dec[b].T),
                "W1": W1,
                "b1": b1,
                "Wfc": Wfc,
            }
        )

    LAST_RESULT = run_bass_kernel_spmd(nc, in_maps, list(range(NCORES)))

    out = np.empty((B, T, U, V), np.float32)
    for c in range(NCORES):
        b, t0 = c // 2, (c % 2) * TC
        out[b, t0 : t0 + TC] = LAST_RESULT.results[c]["out"]
    return out



# revision 26
# speedup vs baseline: 1.1012x; 1.0108x over previous
"""RNN-T JointNet kernel for 8 Trainium2 NeuronCores.

out[b,t,u,:] = gelu_tanh(enc[b,t]@We + dec[b,u]@Wd + b1) @ Wfc

Sharding: flatten (B=4, T=512) -> 2048 rows, 256 contiguous rows per core.
Core c handles batch b=c//2, time slice t0=(c%2)*256 .. +256. Each core only
needs its own enc slice and one batch's dec.

Per-core layout (all fp32):
  - encT (D=256, TC=256), decT (D=256, U=128): host-transposed so the first
    matmuls produce pe/pd with H on partitions, t/u on the free dim.
  - peb[h, t] = enc@We + b1 (4 h-tiles of (128, 256) in SBUF)
  - pd[h, u]  = dec@Wd      (4 h-tiles of (128, 128) in SBUF)
  - main loop over groups of TB=8 t's:
      DVE:  tmp[h, tb, u] = pd[h, u] + peb[h, t0+tb]   (broadcast APs)
      ACT:  hact = gelu_tanh(tmp)
      PE :  out_psum(u=128, v=512) = sum_ht hact[ht][:, tb]ᵀ-block @ Wfc[ht]
            (hact tile is the stationary operand, Wfc streams, N=512)
      DMA:  out_psum -> out[t] (contiguous 256 KiB)
"""

import sys

import numpy as np

sys.path.insert(0, "/opt/trn_rl_repo")

import concourse.bacc as bacc
import concourse.bass as bass
import concourse.mybir as mybir
import concourse.tile as tile
from concourse.bass_utils import run_bass_kernel_spmd

B, T, U, D, H, V = 4, 512, 128, 256, 512, 512
NCORES = 8
TC = (B * T) // NCORES  # 256 t-rows per core
TB = 8  # t's per main-loop group

_PROGRAM = None
LAST_RESULT = None


def _build():
    global _PROGRAM
    if _PROGRAM is not None:
        return _PROGRAM

    f32 = mybir.dt.float32
    bf16 = mybir.dt.bfloat16
    # Bacc (not raw Bass): its compile() pipeline moves matmul waits onto
    # ldweights and splits >1-wait instructions via event semaphores —
    # walrus rejects matmuls carrying 2 sync waits otherwise.
    nc = bacc.Bacc("TRN2", target_bir_lowering=False)

    encT_d = nc.declare_dram_parameter("encT", (D, TC), f32, isOutput=False)
    decT_d = nc.declare_dram_parameter("decT", (D, U), f32, isOutput=False)
    w1_d = nc.declare_dram_parameter("W1", (2 * D, H), f32, isOutput=False)
    b1_d = nc.declare_dram_parameter("b1", (H, 1), f32, isOutput=False)
    wfc_d = nc.declare_dram_parameter("Wfc", (H, V), bf16, isOutput=False)
    out_d = nc.declare_dram_parameter("out", (TC, U, V), f32, isOutput=True)

    GELU = mybir.ActivationFunctionType.Gelu_apprx_tanh
    IDENT = mybir.ActivationFunctionType.Identity
    # Same bits as fp32, but the PE streams it at 1 cycle/row (vs 4 for
    # plain fp32) when the moving free dim is >= 256.
    F32R = mybir.dt.float32r

    with tile.TileContext(nc) as tc:
        with (
            tc.tile_pool(name="const", bufs=1) as cpool,
            tc.tile_pool(name="work", bufs=2) as wpool,
            tc.tile_pool(name="outsb", bufs=2) as osb_pool,
            tc.tile_pool(name="pro_ps", bufs=1, space="PSUM") as pro_ps,
            tc.tile_pool(name="out_ps", bufs=6, space="PSUM") as out_ps_pool,
        ):
            # W1 row-block i (128 rows of the 512-row input dim) lives at
            # cols [i*H, (i+1)*H). Blocks 0,1 = We; blocks 2,3 = Wd.
            # Tiles that only feed matmuls are float32r so the PE streams
            # them at 1 cycle/row; the BIR verifier requires the producer
            # (DMA/ACT) output dtype to be f32r as well.
            w1_sb = cpool.tile([128, 4 * H], F32R)
            wfc_sb = cpool.tile([128, 4 * V], bf16)  # block ht = Wfc[ht*128:...]
            b1_sb = cpool.tile([128, 4], f32)  # col ht = b1[ht*128:(ht+1)*128]
            encT_sb = cpool.tile([128, 2 * TC], F32R)
            decT_sb = cpool.tile([128, 2 * U], F32R)
            peb_sb = cpool.tile([128, 4 * TC], f32)
            pd_sb = cpool.tile([128, 4 * U], f32)

            # One DMA per SBUF tile (3D APs): keeps the per-instruction sync
            # wait count low (walrus rejects >N waits on a matmul) and the
            # transfers large. Ordered so the prologue's first matmuls
            # (enc @ We) can start as early as possible; wfc is not needed
            # until the first main-loop matmul.
            nc.sync.dma_start(
                encT_sb.rearrange("p (i t) -> p i t", i=2),
                encT_d[:, :].rearrange("(i p) t -> p i t", p=128).bitcast(F32R),
            )
            nc.sync.dma_start(
                w1_sb.rearrange("p (i h) -> p i h", i=4),
                w1_d[:, :].rearrange("(i p) h -> p i h", p=128).bitcast(F32R),
            )
            nc.sync.dma_start(
                decT_sb.rearrange("p (i u) -> p i u", i=2),
                decT_d[:, :].rearrange("(i p) u -> p i u", p=128).bitcast(F32R),
            )
            nc.sync.dma_start(
                b1_sb, b1_d[:, :].rearrange("(i p) o -> p (i o)", p=128)
            )
            nc.sync.dma_start(
                wfc_sb.rearrange("p (i v) -> p i v", i=4),
                wfc_d[:, :].rearrange("(i p) v -> p i v", p=128),
            )

            # Prologue: pe[h,t] = enc@We ; pdb[h,u] = dec@Wd + b1. The
            # PSUM->SBUF bounce and the b1 add run on DVE so ACT stays
            # dedicated to the main-loop GELUs.
            for ht in range(4):
                pe_ps = pro_ps.tile([128, TC], f32)
                for di in range(2):
                    nc.tensor.matmul(
                        pe_ps,
                        w1_sb[:, di * H + ht * 128 : di * H + (ht + 1) * 128],
                        encT_sb[:, di * TC : (di + 1) * TC],
                        start=(di == 0),
                        stop=(di == 1),
                    )
                nc.vector.tensor_copy(peb_sb[:, ht * TC : (ht + 1) * TC], pe_ps)
                pd_ps = pro_ps.tile([128, U], f32)
                for di in range(2):
                    nc.tensor.matmul(
                        pd_ps,
                        w1_sb[:, (2 + di) * H + ht * 128 : (2 + di) * H + (ht + 1) * 128],
                        decT_sb[:, di * U : (di + 1) * U],
                        start=(di == 0),
                        stop=(di == 1),
                    )
                nc.vector.tensor_scalar_add(
                    pd_sb[:, ht * U : (ht + 1) * U],
                    pd_ps,
                    b1_sb[:, ht : ht + 1],
                )

            # Main loop over u (in blocks of UB): ACT fuses the pd[:,u] add
            # into the GELU as a per-partition bias; h_u (h on partitions, t
            # free) feeds the PE as the stationary operand; DVE bounces
            # PSUM->SBUF staging; one DMA per (ts, u-block) stores
            # (128 t, UB u, 512 v) slabs — 16 KiB contiguous per t row.
            UB = 8
            for ub in range(U // UB):
                # hact for all UB u's of this block up front; both ts halves
                # consume it, so each stage's DMA can issue mid-block
                # instead of both piling up at the end.
                hblk = []
                for j in range(UB):
                    u = ub * UB + j
                    hts = []
                    for ht in range(4):
                        hact = wpool.tile(
                            [128, TC], bf16, tag=f"h{ht}_{j}", name=f"hact{ht}_{j}"
                        )
                        nc.scalar.activation(
                            hact,
                            peb_sb[:, ht * TC : (ht + 1) * TC],
                            GELU,
                            bias=pd_sb[:, ht * U + u : ht * U + u + 1],
                        )
                        hts.append(hact)
                    hblk.append(hts)
                for ts in range(TC // 128):
                    stage = osb_pool.tile(
                        [128, UB * V], f32, tag=f"st{ts}", name=f"stage{ts}"
                    )
                    for j in range(UB):
                        ops = out_ps_pool.tile([128, V], f32)
                        for ht in range(4):
                            nc.tensor.matmul(
                                ops,
                                hblk[j][ht][:, ts * 128 : (ts + 1) * 128],
                                wfc_sb[:, ht * V : (ht + 1) * V],
                                start=(ht == 0),
                                stop=(ht == 3),
                            )
                        nc.vector.tensor_copy(
                            stage[:, j * V : (j + 1) * V], ops
                        )
                    nc.sync.dma_start(
                        out_d[ts * 128 : (ts + 1) * 128, ub * UB : (ub + 1) * UB, :],
                        stage.rearrange("p (j v) -> p j v", j=UB),
                    )

    nc.compile()
    _PROGRAM = nc
    return nc


def kernel(enc, dec, W1, b1, Wfc):
    global LAST_RESULT
    nc = _build()
    enc = np.asarray(enc, dtype=np.float32)
    dec = np.asarray(dec, dtype=np.float32)
    import ml_dtypes

    W1 = np.ascontiguousarray(np.asarray(W1, dtype=np.float32))
    b1 = np.ascontiguousarray(np.asarray(b1, dtype=np.float32).reshape(H, 1))
    Wfc = np.ascontiguousarray(
        np.asarray(Wfc, dtype=np.float32).astype(ml_dtypes.bfloat16)
    )

    in_maps = []
    for c in range(NCORES):
        b, t0 = c // 2, (c % 2) * TC
        in_maps.append(
            {
                "encT": np.ascontiguousarray(enc[b, t0 : t0 + TC, :].T),
                "decT": np.ascontiguousarray(dec[b].T),
                "W1": W1,
                "b1": b1,
                "Wfc": Wfc,
            }
        )

    LAST_RESULT = run_bass_kernel_spmd(nc, in_maps, list(range(NCORES)))

    out = np.empty((B, T, U, V), np.float32)
    for c in range(NCORES):
        b, t0 = c // 2, (c % 2) * TC
        out[b, t0 : t0 + TC] = LAST_RESULT.results[c]["out"]
    return out



# revision 31
# speedup vs baseline: 1.1085x; 1.0066x over previous
"""RNN-T JointNet kernel for 8 Trainium2 NeuronCores.

out[b,t,u,:] = gelu_tanh(enc[b,t]@We + dec[b,u]@Wd + b1) @ Wfc

Sharding: flatten (B=4, T=512) -> 2048 rows, 256 contiguous rows per core.
Core c handles batch b=c//2, time slice t0=(c%2)*256 .. +256. Each core only
needs its own enc slice and one batch's dec.

Per-core layout (all fp32):
  - encT (D=256, TC=256), decT (D=256, U=128): host-transposed so the first
    matmuls produce pe/pd with H on partitions, t/u on the free dim.
  - peb[h, t] = enc@We + b1 (4 h-tiles of (128, 256) in SBUF)
  - pd[h, u]  = dec@Wd      (4 h-tiles of (128, 128) in SBUF)
  - main loop over groups of TB=8 t's:
      DVE:  tmp[h, tb, u] = pd[h, u] + peb[h, t0+tb]   (broadcast APs)
      ACT:  hact = gelu_tanh(tmp)
      PE :  out_psum(u=128, v=512) = sum_ht hact[ht][:, tb]ᵀ-block @ Wfc[ht]
            (hact tile is the stationary operand, Wfc streams, N=512)
      DMA:  out_psum -> out[t] (contiguous 256 KiB)
"""

import sys

import numpy as np

sys.path.insert(0, "/opt/trn_rl_repo")

import concourse.bacc as bacc
import concourse.bass as bass
import concourse.mybir as mybir
import concourse.tile as tile
from concourse.bass_utils import run_bass_kernel_spmd

B, T, U, D, H, V = 4, 512, 128, 256, 512, 512
NCORES = 8
TC = (B * T) // NCORES  # 256 t-rows per core
TB = 8  # t's per main-loop group

_PROGRAM = None
LAST_RESULT = None


def _build():
    global _PROGRAM
    if _PROGRAM is not None:
        return _PROGRAM

    f32 = mybir.dt.float32
    bf16 = mybir.dt.bfloat16
    # Bacc (not raw Bass): its compile() pipeline moves matmul waits onto
    # ldweights and splits >1-wait instructions via event semaphores —
    # walrus rejects matmuls carrying 2 sync waits otherwise.
    nc = bacc.Bacc("TRN2", target_bir_lowering=False)

    encT_d = nc.declare_dram_parameter("encT", (D, TC), f32, isOutput=False)
    decT_d = nc.declare_dram_parameter("decT", (D, U), f32, isOutput=False)
    w1_d = nc.declare_dram_parameter("W1", (2 * D, H), f32, isOutput=False)
    b1_d = nc.declare_dram_parameter("b1", (H, 1), f32, isOutput=False)
    wfc_d = nc.declare_dram_parameter("Wfc", (H, V), bf16, isOutput=False)
    out_d = nc.declare_dram_parameter("out", (TC, U, V), f32, isOutput=True)

    GELU = mybir.ActivationFunctionType.Gelu_apprx_tanh
    IDENT = mybir.ActivationFunctionType.Identity
    # Same bits as fp32, but the PE streams it at 1 cycle/row (vs 4 for
    # plain fp32) when the moving free dim is >= 256.
    F32R = mybir.dt.float32r

    with tile.TileContext(nc) as tc:
        with (
            tc.tile_pool(name="const", bufs=1) as cpool,
            tc.tile_pool(name="work", bufs=2) as wpool,
            tc.tile_pool(name="outsb", bufs=2) as osb_pool,
            tc.tile_pool(name="pro_ps", bufs=1, space="PSUM") as pro_ps,
            tc.tile_pool(name="out_ps", bufs=6, space="PSUM") as out_ps_pool,
        ):
            # W1 row-block i (128 rows of the 512-row input dim) lives at
            # cols [i*H, (i+1)*H). Blocks 0,1 = We; blocks 2,3 = Wd.
            # Tiles that only feed matmuls are float32r so the PE streams
            # them at 1 cycle/row; the BIR verifier requires the producer
            # (DMA/ACT) output dtype to be f32r as well.
            w1_sb = cpool.tile([128, 4 * H], F32R)
            wfc_sb = cpool.tile([128, 4 * V], bf16)  # block ht = Wfc[ht*128:...]
            b1_sb = cpool.tile([128, 4], f32)  # col ht = b1[ht*128:(ht+1)*128]
            encT_sb = cpool.tile([128, 2 * TC], F32R)
            decT_sb = cpool.tile([128, 2 * U], F32R)
            peb_sb = cpool.tile([128, 4 * TC], f32)
            pd_sb = cpool.tile([128, 4 * U], f32)

            # One DMA per SBUF tile (3D APs): keeps the per-instruction sync
            # wait count low (walrus rejects >N waits on a matmul) and the
            # transfers large. Ordered so the prologue's first matmuls
            # (enc @ We) can start as early as possible; wfc is not needed
            # until the first main-loop matmul.
            nc.sync.dma_start(
                encT_sb.rearrange("p (i t) -> p i t", i=2),
                encT_d[:, :].rearrange("(i p) t -> p i t", p=128).bitcast(F32R),
            )
            # w1 split in two so the enc @ We matmuls start after only the
            # We half (blocks 0,1) has landed.
            nc.sync.dma_start(
                w1_sb[:, : 2 * H].rearrange("p (i h) -> p i h", i=2),
                w1_d[:D, :].rearrange("(i p) h -> p i h", p=128).bitcast(F32R),
            )
            nc.sync.dma_start(
                w1_sb[:, 2 * H :].rearrange("p (i h) -> p i h", i=2),
                w1_d[D:, :].rearrange("(i p) h -> p i h", p=128).bitcast(F32R),
            )
            nc.sync.dma_start(
                decT_sb.rearrange("p (i u) -> p i u", i=2),
                decT_d[:, :].rearrange("(i p) u -> p i u", p=128).bitcast(F32R),
            )
            nc.sync.dma_start(
                b1_sb, b1_d[:, :].rearrange("(i p) o -> p (i o)", p=128)
            )
            nc.sync.dma_start(
                wfc_sb.rearrange("p (i v) -> p i v", i=4),
                wfc_d[:, :].rearrange("(i p) v -> p i v", p=128),
            )

            # Warm the ACT Gelu table during the input-DMA wait so the first
            # real GELU doesn't pay the ~1.3us table load.
            warm_src = cpool.tile([128, 1], f32)
            warm_dst = cpool.tile([128, 1], f32)
            nc.vector.memset(warm_src, 0.0)
            nc.scalar.activation(warm_dst, warm_src, GELU)

            # Prologue: pe[h,t] = enc@We ; pdb[h,u] = dec@Wd + b1. The
            # PSUM->SBUF bounce and the b1 add run on DVE so ACT stays
            # dedicated to the main-loop GELUs.
            for ht in range(4):
                pe_ps = pro_ps.tile([128, TC], f32)
                for di in range(2):
                    nc.tensor.matmul(
                        pe_ps,
                        w1_sb[:, di * H + ht * 128 : di * H + (ht + 1) * 128],
                        encT_sb[:, di * TC : (di + 1) * TC],
                        start=(di == 0),
                        stop=(di == 1),
                    )
                nc.vector.tensor_copy(peb_sb[:, ht * TC : (ht + 1) * TC], pe_ps)
                pd_ps = pro_ps.tile([128, U], f32)
                for di in range(2):
                    nc.tensor.matmul(
                        pd_ps,
                        w1_sb[:, (2 + di) * H + ht * 128 : (2 + di) * H + (ht + 1) * 128],
                        decT_sb[:, di * U : (di + 1) * U],
                        start=(di == 0),
                        stop=(di == 1),
                    )
                nc.vector.tensor_scalar_add(
                    pd_sb[:, ht * U : (ht + 1) * U],
                    pd_ps,
                    b1_sb[:, ht : ht + 1],
                )

            # Main loop over u (in blocks of UB): ACT fuses the pd[:,u] add
            # into the GELU as a per-partition bias; h_u (h on partitions, t
            # free) feeds the PE as the stationary operand; DVE bounces
            # PSUM->SBUF staging; one DMA per (ts, u-block) stores
            # (128 t, UB u, 512 v) slabs — 16 KiB contiguous per t row.
            UB = 8
            for ub in range(U // UB):
                # hact for all UB u's of this block up front; both ts halves
                # consume it, so each stage's DMA can issue mid-block
                # instead of both piling up at the end.
                hblk = []
                for j in range(UB):
                    u = ub * UB + j
                    hts = []
                    for ht in range(4):
                        hact = wpool.tile(
                            [128, TC], bf16, tag=f"h{ht}_{j}", name=f"hact{ht}_{j}"
                        )
                        nc.scalar.activation(
                            hact,
                            peb_sb[:, ht * TC : (ht + 1) * TC],
                            GELU,
                            bias=pd_sb[:, ht * U + u : ht * U + u + 1],
                        )
                        hts.append(hact)
                    hblk.append(hts)
                last_ub = ub == U // UB - 1
                for ts in range(TC // 128):
                    stage = osb_pool.tile(
                        [128, UB * V], f32, tag=f"st{ts}", name=f"stage{ts}"
                    )
                    for j in range(UB):
                        ops = out_ps_pool.tile([128, V], f32)
                        for ht in range(4):
                            nc.tensor.matmul(
                                ops,
                                hblk[j][ht][:, ts * 128 : (ts + 1) * 128],
                                wfc_sb[:, ht * V : (ht + 1) * V],
                                start=(ht == 0),
                                stop=(ht == 3),
                            )
                        nc.vector.tensor_copy(
                            stage[:, j * V : (j + 1) * V], ops
                        )
                        # On the last block, stream the stores out in 2-u
                        # chunks right behind the copies so the final DMA
                        # drain after the last matmul is short.
                        if last_ub and j % 2 == 1:
                            nc.sync.dma_start(
                                out_d[
                                    ts * 128 : (ts + 1) * 128,
                                    ub * UB + j - 1 : ub * UB + j + 1,
                                    :,
                                ],
                                stage[:, (j - 1) * V : (j + 1) * V].rearrange(
                                    "p (j v) -> p j v", j=2
                                ),
                            )
                    if not last_ub:
                        nc.sync.dma_start(
                            out_d[ts * 128 : (ts + 1) * 128, ub * UB : (ub + 1) * UB, :],
                            stage.rearrange("p (j v) -> p j v", j=UB),
                        )

    nc.compile()
    _PROGRAM = nc
    return nc


def kernel(enc, dec, W1, b1, Wfc):
    global LAST_RESULT
    nc = _build()
    enc = np.asarray(enc, dtype=np.float32)
    dec = np.asarray(dec, dtype=np.float32)
    import ml_dtypes

    W1 = np.ascontiguousarray(np.asarray(W1, dtype=np.float32))
    b1 = np.ascontiguousarray(np.asarray(b1, dtype=np.float32).reshape(H, 1))
    Wfc = np.ascontiguousarray(
        np.asarray(Wfc, dtype=np.float32).astype(ml_dtypes.bfloat16)
    )

    in_maps = []
    for c in range(NCORES):
        b, t0 = c // 2, (c % 2) * TC
        in_maps.append(
            {
                "encT": np.ascontiguousarray(enc[b, t0 : t0 + TC, :].T),
                "decT": np.ascontiguousarray(dec[b].T),
                "W1": W1,
                "b1": b1,
                "Wfc": Wfc,
            }
        )

    LAST_RESULT = run_bass_kernel_spmd(nc, in_maps, list(range(NCORES)))

    out = np.empty((B, T, U, V), np.float32)
    for c in range(NCORES):
        b, t0 = c // 2, (c % 2) * TC
        out[b, t0 : t0 + TC] = LAST_RESULT.results[c]["out"]
    return out



# revision 33
# speedup vs baseline: 1.1167x; 1.0074x over previous
"""RNN-T JointNet kernel for 8 Trainium2 NeuronCores.

out[b,t,u,:] = gelu_tanh(enc[b,t]@We + dec[b,u]@Wd + b1) @ Wfc

Sharding: flatten (B=4, T=512) -> 2048 rows, 256 contiguous rows per core.
Core c handles batch b=c//2, time slice t0=(c%2)*256 .. +256. Each core only
needs its own enc slice and one batch's dec.

Per-core layout (all fp32):
  - encT (D=256, TC=256), decT (D=256, U=128): host-transposed so the first
    matmuls produce pe/pd with H on partitions, t/u on the free dim.
  - peb[h, t] = enc@We + b1 (4 h-tiles of (128, 256) in SBUF)
  - pd[h, u]  = dec@Wd      (4 h-tiles of (128, 128) in SBUF)
  - main loop over groups of TB=8 t's:
      DVE:  tmp[h, tb, u] = pd[h, u] + peb[h, t0+tb]   (broadcast APs)
      ACT:  hact = gelu_tanh(tmp)
      PE :  out_psum(u=128, v=512) = sum_ht hact[ht][:, tb]ᵀ-block @ Wfc[ht]
            (hact tile is the stationary operand, Wfc streams, N=512)
      DMA:  out_psum -> out[t] (contiguous 256 KiB)
"""

import sys

import numpy as np

sys.path.insert(0, "/opt/trn_rl_repo")

import concourse.bacc as bacc
import concourse.bass as bass
import concourse.mybir as mybir
import concourse.tile as tile
from concourse.bass_utils import run_bass_kernel_spmd

B, T, U, D, H, V = 4, 512, 128, 256, 512, 512
NCORES = 8
TC = (B * T) // NCORES  # 256 t-rows per core
TB = 8  # t's per main-loop group

_PROGRAM = None
LAST_RESULT = None


def _build():
    global _PROGRAM
    if _PROGRAM is not None:
        return _PROGRAM

    f32 = mybir.dt.float32
    bf16 = mybir.dt.bfloat16
    # Bacc (not raw Bass): its compile() pipeline moves matmul waits onto
    # ldweights and splits >1-wait instructions via event semaphores —
    # walrus rejects matmuls carrying 2 sync waits otherwise.
    nc = bacc.Bacc("TRN2", target_bir_lowering=False)

    encT_d = nc.declare_dram_parameter("encT", (D, TC), f32, isOutput=False)
    decT_d = nc.declare_dram_parameter("decT", (D, U), f32, isOutput=False)
    w1_d = nc.declare_dram_parameter("W1", (2 * D, H), f32, isOutput=False)
    b1_d = nc.declare_dram_parameter("b1", (H, 1), f32, isOutput=False)
    wfc_d = nc.declare_dram_parameter("Wfc", (H, V), bf16, isOutput=False)
    out_d = nc.declare_dram_parameter("out", (TC, U, V), f32, isOutput=True)

    GELU = mybir.ActivationFunctionType.Gelu_apprx_tanh
    IDENT = mybir.ActivationFunctionType.Identity
    # Same bits as fp32, but the PE streams it at 1 cycle/row (vs 4 for
    # plain fp32) when the moving free dim is >= 256.
    F32R = mybir.dt.float32r

    with tile.TileContext(nc) as tc:
        with (
            tc.tile_pool(name="const", bufs=1) as cpool,
            tc.tile_pool(name="work", bufs=3) as wpool,
            tc.tile_pool(name="outsb", bufs=2) as osb_pool,
            tc.tile_pool(name="pro_ps", bufs=1, space="PSUM") as pro_ps,
            tc.tile_pool(name="out_ps", bufs=6, space="PSUM") as out_ps_pool,
        ):
            # W1 row-block i (128 rows of the 512-row input dim) lives at
            # cols [i*H, (i+1)*H). Blocks 0,1 = We; blocks 2,3 = Wd.
            # Tiles that only feed matmuls are float32r so the PE streams
            # them at 1 cycle/row; the BIR verifier requires the producer
            # (DMA/ACT) output dtype to be f32r as well.
            w1_sb = cpool.tile([128, 4 * H], F32R)
            wfc_sb = cpool.tile([128, 4 * V], bf16)  # block ht = Wfc[ht*128:...]
            b1_sb = cpool.tile([128, 4], f32)  # col ht = b1[ht*128:(ht+1)*128]
            encT_sb = cpool.tile([128, 2 * TC], F32R)
            decT_sb = cpool.tile([128, 2 * U], F32R)
            peb_sb = cpool.tile([128, 4 * TC], f32)
            pd_sb = cpool.tile([128, 4 * U], f32)

            # One DMA per SBUF tile (3D APs): keeps the per-instruction sync
            # wait count low (walrus rejects >N waits on a matmul) and the
            # transfers large. Ordered so the prologue's first matmuls
            # (enc @ We) can start as early as possible; wfc is not needed
            # until the first main-loop matmul.
            nc.sync.dma_start(
                encT_sb.rearrange("p (i t) -> p i t", i=2),
                encT_d[:, :].rearrange("(i p) t -> p i t", p=128).bitcast(F32R),
            )
            # w1 split in two so the enc @ We matmuls start after only the
            # We half (blocks 0,1) has landed.
            nc.sync.dma_start(
                w1_sb[:, : 2 * H].rearrange("p (i h) -> p i h", i=2),
                w1_d[:D, :].rearrange("(i p) h -> p i h", p=128).bitcast(F32R),
            )
            nc.sync.dma_start(
                w1_sb[:, 2 * H :].rearrange("p (i h) -> p i h", i=2),
                w1_d[D:, :].rearrange("(i p) h -> p i h", p=128).bitcast(F32R),
            )
            nc.sync.dma_start(
                decT_sb.rearrange("p (i u) -> p i u", i=2),
                decT_d[:, :].rearrange("(i p) u -> p i u", p=128).bitcast(F32R),
            )
            nc.sync.dma_start(
                b1_sb, b1_d[:, :].rearrange("(i p) o -> p (i o)", p=128)
            )
            nc.sync.dma_start(
                wfc_sb.rearrange("p (i v) -> p i v", i=4),
                wfc_d[:, :].rearrange("(i p) v -> p i v", p=128),
            )

            # Warm the ACT Gelu table during the input-DMA wait so the first
            # real GELU doesn't pay the ~1.3us table load.
            warm_src = cpool.tile([128, 1], f32)
            warm_dst = cpool.tile([128, 1], f32)
            nc.vector.memset(warm_src, 0.0)
            nc.scalar.activation(warm_dst, warm_src, GELU)

            # Prologue: pe[h,t] = enc@We ; pdb[h,u] = dec@Wd + b1. The
            # PSUM->SBUF bounce and the b1 add run on DVE so ACT stays
            # dedicated to the main-loop GELUs.
            for ht in range(4):
                pe_ps = pro_ps.tile([128, TC], f32)
                for di in range(2):
                    nc.tensor.matmul(
                        pe_ps,
                        w1_sb[:, di * H + ht * 128 : di * H + (ht + 1) * 128],
                        encT_sb[:, di * TC : (di + 1) * TC],
                        start=(di == 0),
                        stop=(di == 1),
                    )
                nc.vector.tensor_copy(peb_sb[:, ht * TC : (ht + 1) * TC], pe_ps)
                pd_ps = pro_ps.tile([128, U], f32)
                for di in range(2):
                    nc.tensor.matmul(
                        pd_ps,
                        w1_sb[:, (2 + di) * H + ht * 128 : (2 + di) * H + (ht + 1) * 128],
                        decT_sb[:, di * U : (di + 1) * U],
                        start=(di == 0),
                        stop=(di == 1),
                    )
                nc.vector.tensor_scalar_add(
                    pd_sb[:, ht * U : (ht + 1) * U],
                    pd_ps,
                    b1_sb[:, ht : ht + 1],
                )

            # Main loop over u (in blocks of UB): ACT fuses the pd[:,u] add
            # into the GELU as a per-partition bias; h_u (h on partitions, t
            # free) feeds the PE as the stationary operand; DVE bounces
            # PSUM->SBUF staging; one DMA per (ts, u-block) stores
            # (128 t, UB u, 512 v) slabs — 16 KiB contiguous per t row.
            UB = 8
            for ub in range(U // UB):
                # hact for all UB u's of this block up front; both ts halves
                # consume it, so each stage's DMA can issue mid-block
                # instead of both piling up at the end.
                hblk = []
                for j in range(UB):
                    u = ub * UB + j
                    hts = []
                    for ht in range(4):
                        hact = wpool.tile(
                            [128, TC], bf16, tag=f"h{ht}_{j}", name=f"hact{ht}_{j}"
                        )
                        nc.scalar.activation(
                            hact,
                            peb_sb[:, ht * TC : (ht + 1) * TC],
                            GELU,
                            bias=pd_sb[:, ht * U + u : ht * U + u + 1],
                        )
                        hts.append(hact)
                    hblk.append(hts)
                last_ub = ub == U // UB - 1
                for ts in range(TC // 128):
                    stage = osb_pool.tile(
                        [128, UB * V], f32, tag=f"st{ts}", name=f"stage{ts}"
                    )
                    for j in range(UB):
                        ops = out_ps_pool.tile([128, V], f32)
                        for ht in range(4):
                            nc.tensor.matmul(
                                ops,
                                hblk[j][ht][:, ts * 128 : (ts + 1) * 128],
                                wfc_sb[:, ht * V : (ht + 1) * V],
                                start=(ht == 0),
                                stop=(ht == 3),
                            )
                        nc.vector.tensor_copy(
                            stage[:, j * V : (j + 1) * V], ops
                        )
                        # On the last block, stream the stores out in small
                        # chunks right behind the copies so the final DMA
                        # drain after the last matmul is short: 2-u chunks,
                        # then 1-u for the final half of ts=1.
                        if last_ub:
                            fine = ts == 1 and j >= UB // 2
                            if fine:
                                nc.sync.dma_start(
                                    out_d[
                                        ts * 128 : (ts + 1) * 128,
                                        ub * UB + j : ub * UB + j + 1,
                                        :,
                                    ],
                                    stage[:, j * V : (j + 1) * V][:, None, :],
                                )
                            elif j % 2 == 1:
                                nc.sync.dma_start(
                                    out_d[
                                        ts * 128 : (ts + 1) * 128,
                                        ub * UB + j - 1 : ub * UB + j + 1,
                                        :,
                                    ],
                                    stage[:, (j - 1) * V : (j + 1) * V].rearrange(
                                        "p (j v) -> p j v", j=2
                                    ),
                                )
                    if not last_ub:
                        nc.sync.dma_start(
                            out_d[ts * 128 : (ts + 1) * 128, ub * UB : (ub + 1) * UB, :],
                            stage.rearrange("p (j v) -> p j v", j=UB),
                        )

    nc.compile()
    _PROGRAM = nc
    return nc


def kernel(enc, dec, W1, b1, Wfc):
    global LAST_RESULT
    nc = _build()
    enc = np.asarray(enc, dtype=np.float32)
    dec = np.asarray(dec, dtype=np.float32)
    import ml_dtypes

    W1 = np.ascontiguousarray(np.asarray(W1, dtype=np.float32))
    b1 = np.ascontiguousarray(np.asarray(b1, dtype=np.float32).reshape(H, 1))
    Wfc = np.ascontiguousarray(
        np.asarray(Wfc, dtype=np.float32).astype(ml_dtypes.bfloat16)
    )

    in_maps = []
    for c in range(NCORES):
        b, t0 = c // 2, (c % 2) * TC
        in_maps.append(
            {
                "encT": np.ascontiguousarray(enc[b, t0 : t0 + TC, :].T),
                "decT": np.ascontiguousarray(dec[b].T),
                "W1": W1,
                "b1": b1,
                "Wfc": Wfc,
            }
        )

    LAST_RESULT = run_bass_kernel_spmd(nc, in_maps, list(range(NCORES)))

    out = np.empty((B, T, U, V), np.float32)
    for c in range(NCORES):
        b, t0 = c // 2, (c % 2) * TC
        out[b, t0 : t0 + TC] = LAST_RESULT.results[c]["out"]
    return out



# revision 37
# speedup vs baseline: 1.1333x; 1.0149x over previous
"""RNN-T JointNet kernel for 8 Trainium2 NeuronCores.

out[b,t,u,:] = gelu_tanh(enc[b,t]@We + dec[b,u]@Wd + b1) @ Wfc

Sharding: flatten (B=4, T=512) -> 2048 rows, 256 contiguous rows per core.
Core c handles batch b=c//2, time slice t0=(c%2)*256 .. +256. Each core only
needs its own enc slice and one batch's dec.

Per-core layout (all fp32):
  - encT (D=256, TC=256), decT (D=256, U=128): host-transposed so the first
    matmuls produce pe/pd with H on partitions, t/u on the free dim.
  - peb[h, t] = enc@We + b1 (4 h-tiles of (128, 256) in SBUF)
  - pd[h, u]  = dec@Wd      (4 h-tiles of (128, 128) in SBUF)
  - main loop over groups of TB=8 t's:
      DVE:  tmp[h, tb, u] = pd[h, u] + peb[h, t0+tb]   (broadcast APs)
      ACT:  hact = gelu_tanh(tmp)
      PE :  out_psum(u=128, v=512) = sum_ht hact[ht][:, tb]ᵀ-block @ Wfc[ht]
            (hact tile is the stationary operand, Wfc streams, N=512)
      DMA:  out_psum -> out[t] (contiguous 256 KiB)
"""

import sys

import numpy as np

sys.path.insert(0, "/opt/trn_rl_repo")

import concourse.bacc as bacc
import concourse.bass as bass
import concourse.mybir as mybir
import concourse.tile as tile
from concourse.bass_utils import run_bass_kernel_spmd

B, T, U, D, H, V = 4, 512, 128, 256, 512, 512
NCORES = 8
TC = (B * T) // NCORES  # 256 t-rows per core
TB = 8  # t's per main-loop group

_PROGRAM = None
LAST_RESULT = None


def _build():
    global _PROGRAM
    if _PROGRAM is not None:
        return _PROGRAM

    f32 = mybir.dt.float32
    bf16 = mybir.dt.bfloat16
    # Bacc (not raw Bass): its compile() pipeline moves matmul waits onto
    # ldweights and splits >1-wait instructions via event semaphores —
    # walrus rejects matmuls carrying 2 sync waits otherwise.
    nc = bacc.Bacc("TRN2", target_bir_lowering=False)

    encT_d = nc.declare_dram_parameter("encT", (D, TC), bf16, isOutput=False)
    decT_d = nc.declare_dram_parameter("decT", (D, U), bf16, isOutput=False)
    w1_d = nc.declare_dram_parameter("W1", (2 * D, H), bf16, isOutput=False)
    b1_d = nc.declare_dram_parameter("b1", (H, 1), f32, isOutput=False)
    wfc_d = nc.declare_dram_parameter("Wfc", (H, V), bf16, isOutput=False)
    out_d = nc.declare_dram_parameter("out", (TC, U, V), f32, isOutput=True)

    GELU = mybir.ActivationFunctionType.Gelu_apprx_tanh

    with tile.TileContext(nc) as tc:
        with (
            tc.tile_pool(name="const", bufs=1) as cpool,
            tc.tile_pool(name="work", bufs=3) as wpool,
            tc.tile_pool(name="outsb", bufs=2) as osb_pool,
            tc.tile_pool(name="pro_ps", bufs=1, space="PSUM") as pro_ps,
            tc.tile_pool(name="out_ps", bufs=6, space="PSUM") as out_ps_pool,
        ):
            # W1 row-block i (128 rows of the 512-row input dim) lives at
            # cols [i*H, (i+1)*H). Blocks 0,1 = We; blocks 2,3 = Wd.
            # All matmul operands are bf16 (host-converted): the PE streams
            # bf16 at 1 cycle/row at any width and stationary loads are 2x
            # faster than 4-byte dtypes.
            w1_sb = cpool.tile([128, 4 * H], bf16)
            wfc_sb = cpool.tile([128, 4 * V], bf16)  # block ht = Wfc[ht*128:...]
            b1_sb = cpool.tile([128, 4], f32)  # col ht = b1[ht*128:(ht+1)*128]
            encT_sb = cpool.tile([128, 2 * TC], bf16)
            decT_sb = cpool.tile([128, 2 * U], bf16)
            peb_sb = cpool.tile([128, 4 * TC], f32)
            pd_sb = cpool.tile([128, 4 * U], f32)

            # One DMA per SBUF tile (3D APs): keeps the per-instruction sync
            # wait count low (walrus rejects >N waits on a matmul) and the
            # transfers large. Ordered so the prologue's first matmuls
            # (enc @ We) can start as early as possible; wfc is not needed
            # until the first main-loop matmul.
            nc.sync.dma_start(
                encT_sb.rearrange("p (i t) -> p i t", i=2),
                encT_d[:, :].rearrange("(i p) t -> p i t", p=128),
            )
            # w1 split in two so the enc @ We matmuls start after only the
            # We half (blocks 0,1) has landed.
            nc.sync.dma_start(
                w1_sb[:, : 2 * H].rearrange("p (i h) -> p i h", i=2),
                w1_d[:D, :].rearrange("(i p) h -> p i h", p=128),
            )
            nc.sync.dma_start(
                w1_sb[:, 2 * H :].rearrange("p (i h) -> p i h", i=2),
                w1_d[D:, :].rearrange("(i p) h -> p i h", p=128),
            )
            nc.sync.dma_start(
                decT_sb.rearrange("p (i u) -> p i u", i=2),
                decT_d[:, :].rearrange("(i p) u -> p i u", p=128),
            )
            nc.sync.dma_start(
                b1_sb, b1_d[:, :].rearrange("(i p) o -> p (i o)", p=128)
            )
            nc.sync.dma_start(
                wfc_sb.rearrange("p (i v) -> p i v", i=4),
                wfc_d[:, :].rearrange("(i p) v -> p i v", p=128),
            )

            # Warm the ACT Gelu table during the input-DMA wait so the first
            # real GELU doesn't pay the ~1.3us table load.
            warm_src = cpool.tile([128, 1], f32)
            warm_dst = cpool.tile([128, 1], f32)
            nc.vector.memset(warm_src, 0.0)
            nc.scalar.activation(warm_dst, warm_src, GELU)

            # Prologue: pe[h,t] = enc@We ; pdb[h,u] = dec@Wd + b1. The
            # PSUM->SBUF bounce and the b1 add run on DVE so ACT stays
            # dedicated to the main-loop GELUs.
            for ht in range(4):
                pe_ps = pro_ps.tile([128, TC], f32)
                for di in range(2):
                    nc.tensor.matmul(
                        pe_ps,
                        w1_sb[:, di * H + ht * 128 : di * H + (ht + 1) * 128],
                        encT_sb[:, di * TC : (di + 1) * TC],
                        start=(di == 0),
                        stop=(di == 1),
                    )
                nc.vector.tensor_copy(peb_sb[:, ht * TC : (ht + 1) * TC], pe_ps)
                pd_ps = pro_ps.tile([128, U], f32)
                for di in range(2):
                    nc.tensor.matmul(
                        pd_ps,
                        w1_sb[:, (2 + di) * H + ht * 128 : (2 + di) * H + (ht + 1) * 128],
                        decT_sb[:, di * U : (di + 1) * U],
                        start=(di == 0),
                        stop=(di == 1),
                    )
                nc.vector.tensor_scalar_add(
                    pd_sb[:, ht * U : (ht + 1) * U],
                    pd_ps,
                    b1_sb[:, ht : ht + 1],
                )

            # Main loop over u (in blocks of UB): ACT fuses the pd[:,u] add
            # into the GELU as a per-partition bias; h_u (h on partitions, t
            # free) feeds the PE as the stationary operand; DVE bounces
            # PSUM->SBUF staging; one DMA per (ts, u-block) stores
            # (128 t, UB u, 512 v) slabs — 16 KiB contiguous per t row.
            UB = 8
            for ub in range(U // UB):
                # hact for all UB u's of this block up front; both ts halves
                # consume it, so each stage's DMA can issue mid-block
                # instead of both piling up at the end.
                hblk = []
                for j in range(UB):
                    u = ub * UB + j
                    hts = []
                    for ht in range(4):
                        hact = wpool.tile(
                            [128, TC], bf16, tag=f"h{ht}_{j}", name=f"hact{ht}_{j}"
                        )
                        nc.scalar.activation(
                            hact,
                            peb_sb[:, ht * TC : (ht + 1) * TC],
                            GELU,
                            bias=pd_sb[:, ht * U + u : ht * U + u + 1],
                        )
                        hts.append(hact)
                    hblk.append(hts)
                last_ub = ub == U // UB - 1
                for ts in range(TC // 128):
                    stage = osb_pool.tile(
                        [128, UB * V], f32, tag=f"st{ts}", name=f"stage{ts}"
                    )
                    for j in range(UB):
                        ops = out_ps_pool.tile([128, V], f32)
                        for ht in range(4):
                            nc.tensor.matmul(
                                ops,
                                hblk[j][ht][:, ts * 128 : (ts + 1) * 128],
                                wfc_sb[:, ht * V : (ht + 1) * V],
                                start=(ht == 0),
                                stop=(ht == 3),
                            )
                        nc.vector.tensor_copy(
                            stage[:, j * V : (j + 1) * V], ops
                        )
                        # On the last block, stream the stores out in small
                        # chunks right behind the copies so the final DMA
                        # drain after the last matmul is short: 2-u chunks,
                        # then 1-u for the final half of ts=1.
                        if last_ub:
                            fine = ts == 1 and j >= UB // 2
                            if fine:
                                nc.sync.dma_start(
                                    out_d[
                                        ts * 128 : (ts + 1) * 128,
                                        ub * UB + j : ub * UB + j + 1,
                                        :,
                                    ],
                                    stage[:, j * V : (j + 1) * V][:, None, :],
                                )
                            elif j % 2 == 1:
                                nc.sync.dma_start(
                                    out_d[
                                        ts * 128 : (ts + 1) * 128,
                                        ub * UB + j - 1 : ub * UB + j + 1,
                                        :,
                                    ],
                                    stage[:, (j - 1) * V : (j + 1) * V].rearrange(
                                        "p (j v) -> p j v", j=2
                                    ),
                                )
                    if not last_ub:
                        nc.sync.dma_start(
                            out_d[ts * 128 : (ts + 1) * 128, ub * UB : (ub + 1) * UB, :],
                            stage.rearrange("p (j v) -> p j v", j=UB),
                        )

    nc.compile()
    _PROGRAM = nc
    return nc


def kernel(enc, dec, W1, b1, Wfc):
    global LAST_RESULT
    nc = _build()
    import ml_dtypes

    bf = ml_dtypes.bfloat16
    enc = np.asarray(enc, dtype=np.float32)
    dec = np.asarray(dec, dtype=np.float32)
    W1 = np.ascontiguousarray(np.asarray(W1, dtype=np.float32).astype(bf))
    b1 = np.ascontiguousarray(np.asarray(b1, dtype=np.float32).reshape(H, 1))
    Wfc = np.ascontiguousarray(np.asarray(Wfc, dtype=np.float32).astype(bf))

    in_maps = []
    for c in range(NCORES):
        b, t0 = c // 2, (c % 2) * TC
        in_maps.append(
            {
                "encT": np.ascontiguousarray(enc[b, t0 : t0 + TC, :].T.astype(bf)),
                "decT": np.ascontiguousarray(dec[b].T.astype(bf)),
                "W1": W1,
                "b1": b1,
                "Wfc": Wfc,
            }
        )

    LAST_RESULT = run_bass_kernel_spmd(nc, in_maps, list(range(NCORES)))

    out = np.empty((B, T, U, V), np.float32)
    for c in range(NCORES):
        b, t0 = c // 2, (c % 2) * TC
        out[b, t0 : t0 + TC] = LAST_RESULT.results[c]["out"]
    return out

